# revision 7
# baseline (speedup 1.0000x reference)
"""Multi-head causal attention with RoPE on 8 Trainium2 NeuronCores — v4.

Sharding: core c -> (batch b = c//4, head-group g = c%4, heads 4g..4g+4).
wq/wk/wv column-sharded by head, wo row-sharded; attention fully local.
Host sums the 4 per-core partial output projections per batch.

Schedule: all pools are allocated once (slot rotation + subtile WAR deps let
consecutive in-program reps pipeline — the graded metric is marginal rep
time). Each rep: project k0/q0 (PE) + RoPE (DVE), then a single 16-iteration
attention sweep per head pair. Per k-tile mi the two heads' QK matmuls write
one [128, 2, 512] PSUM tile on disjoint PE row groups / PSUM banks (they
co-execute on HW), and ONE activation instr exponentiates both heads' scores
into fp16 [128, 2, W] pair-tiles. Remaining work (v/k1/q1 projections in
pair 0, output projection in pair 1) is spread 1-2 items per mi so the PE
always has exp-independent work while Act catches up.

Softmax: denominator via an appended ones-column of v; DVE reciprocal
[1,512] -> partition broadcast by a DMA bounce through a DRAM scratch tile
(a DRAM source AP may replicate across partitions; custom gpsimd ucode like
InstPartitionBroadcast is NOT loaded on this target) -> DVE multiplies the
PV PSUM by the broadcast during unload (fused normalize). pv / recip+bcast /
normalize / out_proj are staggered across mi points (4jg+4 / 4jg+6 / ...)
so the PE never waits on the DVE/DMA chain; the pair-1 tail interleaves
out_proj tiles into the group-3 recip/broadcast latency. Pair 1's early
(exp-throttled) iterations compute the NEXT rep's k0/q0 projections, and
the next rep's xt is prefetched at pair-0 mi 14 — the rep seam carries no
PE stall.

Numerics: matmul operands fp16, accumulation fp32 (PSUM), RoPE trig fp16.
exp uses a constant -2.5 bias (cancels in the softmax normalization).
"""
import sys
sys.path.insert(0, "/opt/trn_rl_repo")
import numpy as np

import concourse.bass as bass
import concourse.tile as tile
from concourse import bacc, mybir
from concourse.bass_utils import run_bass_kernel_spmd

F = mybir.ActivationFunctionType
A = mybir.AluOpType
FP32 = mybir.dt.float32
FP16 = mybir.dt.float16

B, D, H = 2, 1024, 16
NCORES = 8
GROUPS = 4            # head groups (cores per batch)
HL = H // GROUPS      # heads per core = 4
DK = D // H           # 64
JL = HL * DK          # local projection width = 256
ROPE_THETA = 10000.0


def build_mha(S: int, max_phase: int = 9, reps: int = 1):
    """One SPMD program: per-core shard of the full MHA layer."""
    assert S % 512 == 0
    NT = S // 128          # 128-tiles along sequence
    NP = NT // 2           # exp pair tiles
    NC = S // 512          # 512-chunks along sequence
    KT = D // 128          # 8 contraction tiles for projections
    ECH = D // 512
    SCALE = 1.0 / np.sqrt(DK)

    nc = bacc.Bacc(None, target_bir_lowering=False, debug=False)

    xt_in = nc.declare_dram_parameter("xt", [D, S], FP16, isOutput=False)
    wq_in = nc.declare_dram_parameter("wqt", [D, JL], FP16, isOutput=False)
    wk_in = nc.declare_dram_parameter("wkt", [D, JL], FP16, isOutput=False)
    wv_in = nc.declare_dram_parameter("wvt", [D, JL], FP16, isOutput=False)
    wo_in = nc.declare_dram_parameter("wot", [JL, D], FP16, isOutput=False)
    cos_in = nc.declare_dram_parameter("cos64", [DK, S], FP16, isOutput=False)
    sin_in = nc.declare_dram_parameter("sinalt64", [DK, S], FP16, isOutput=False)
    y_out = nc.declare_dram_parameter("y", [S, D], FP16, isOutput=True)

    with tile.TileContext(nc) as tc:
        # ---------------- persistent tensors ----------------
        persist = tc.alloc_tile_pool(name="persist", bufs=1)
        qTb = [persist.tile([128, S], FP16, tag=f"qTb{i}", name=f"qTb{i}") for i in range(2)]
        kTb = [persist.tile([128, S], FP16, tag=f"kTb{i}", name=f"kTb{i}") for i in range(2)]
        # v: [p, pair, sub, head, 65]; cols 0..63 v dims, col 64 ones
        v_sb = persist.tile([128, NP, 2, HL, DK + 1], FP16, tag="v")
        attnT = [persist.tile([128, S], FP16, tag=f"aT{i}", name=f"aT{i}") for i in range(2)]
        woTb = persist.tile([128, 2, D], FP16, tag="woTb")
        cos128 = persist.tile([128, S], FP16, tag="cos128")
        sinalt128 = persist.tile([128, S], FP16, tag="sinalt128")
        xtb = persist.tile([128, KT, S], FP16, tag="xtb")
        wb = {}
        for name, win in (("v", wv_in), ("k", wk_in), ("q", wq_in)):
            wb[name] = persist.tile([128, KT, JL], FP16, tag=f"wb{name}", name=f"wb{name}")
            nc.sync.dma_start(out=wb[name],
                              in_=win[:, :].rearrange("(k p) j -> p k j", p=128))
        nc.sync.dma_start(out=cos128[0:DK, :], in_=cos_in[:, :])
        nc.sync.dma_start(out=cos128[DK:128, :], in_=cos_in[:, :])
        nc.sync.dma_start(out=sinalt128[0:DK, :], in_=sin_in[:, :])
        nc.sync.dma_start(out=sinalt128[DK:128, :], in_=sin_in[:, :])
        nc.sync.dma_start(out=woTb, in_=wo_in[:, :].rearrange("(t p) e -> p t e", p=128))
        nc.vector.memset(v_sb[:, :, :, :, DK:DK + 1], 1.0)
        # exp bias (softmax shift; cancels in the normalization)
        expbias = persist.tile([128, 1], FP32, tag="expbias")
        nc.vector.memset(expbias, -2.5)

        # ---------------- rep-scoped pools (hoisted: slots rotate) ----------
        ropep = tc.alloc_tile_pool(name="ropep", bufs=2)
        es_pool = tc.alloc_tile_pool(name="es", bufs=1)
        den_pool = tc.alloc_tile_pool(name="den", bufs=2)
        dend_pool = tc.alloc_tile_pool(name="dend", bufs=2, space="DRAM")
        bc_pool = tc.alloc_tile_pool(name="bc", bufs=2)
        ysb = tc.alloc_tile_pool(name="ysb", bufs=4)
        sp_pool = tc.alloc_tile_pool(name="sp_ps", bufs=2, space="PSUM")
        ov_pool = tc.alloc_tile_pool(name="ov_ps", bufs=1, space="PSUM")
        aux_pool = tc.alloc_tile_pool(name="aux_ps", bufs=2, space="PSUM")

        def xtb_dma():
            for sc in range(NC):
                nc.sync.dma_start(
                    out=xtb[:, :, 512 * sc:512 * (sc + 1)],
                    in_=xt_in[:, 512 * sc:512 * (sc + 1)].rearrange(
                        "(k p) s -> p k s", p=128))

        for _rep in range(reps):
            rope_state = {}

            def proj_chunk(name, jt, sc):
                key = (name, jt)
                if key not in rope_state:
                    rope_state[key] = (
                        ropep.tile([128, S], FP16, tag="t16", name=f"t16_{name}{jt}"),
                        ropep.tile([128, S], FP16, tag="swp", name=f"swp_{name}{jt}"),
                        ropep.tile([128, S], FP16, tag="ropetmp", name=f"tmp_{name}{jt}"),
                    )
                t16 = rope_state[key][0]
                ps = aux_pool.tile([128, 512], FP32, tag="aux",
                                   name=f"ps_{name}{jt}_{sc}")
                for k in range(KT):
                    nc.tensor.matmul(
                        out=ps,
                        lhsT=wb[name][:, k, 128 * jt:128 * (jt + 1)],
                        rhs=xtb[:, k, 512 * sc:512 * (sc + 1)],
                        start=(k == 0), stop=(k == KT - 1))
                nc.vector.tensor_copy(out=t16[:, 512 * sc:512 * (sc + 1)], in_=ps)

            def proj_rope(name, jt):
                # RoPE: perm layout (per 64-row head block: 32 even-d then odd-d)
                t16, swp, tmp = rope_state.pop((name, jt))
                dst = (kTb if name == "k" else qTb)[jt]
                for blk in range(4):
                    src_b, dst_b = 32 * (blk ^ 1), 32 * blk
                    nc.sync.dma_start(out=swp[dst_b:dst_b + 32, :],
                                      in_=t16[src_b:src_b + 32, :])
                nc.vector.tensor_mul(tmp, t16, cos128)
                nc.vector.tensor_mul(swp, swp, sinalt128)
                nc.vector.tensor_add(dst, tmp, swp)

            def project_v(st):
                ps = aux_pool.tile([128, 512], FP32, tag="aux", name=f"psv_{st}")
                for k in range(KT):
                    nc.tensor.matmul(out=ps[:, 0:JL],
                                     lhsT=xtb[:, k, 128 * st:128 * (st + 1)],
                                     rhs=wb["v"][:, k, :],
                                     start=(k == 0), stop=(k == KT - 1))
                nc.vector.tensor_copy(
                    out=v_sb[:, st // 2, st % 2, :, 0:DK],
                    in_=ps[:, 0:JL].rearrange("p (h d) -> p h d", h=HL))

            def out_proj_st(st, act_unload=False):
                yst = ysb.tile([128, D], FP16, tag="yst", name=f"yst{st}")
                for ec in range(ECH):
                    po = aux_pool.tile([128, 512], FP32, tag="aux",
                                       name=f"po{st}_{ec}")
                    for jt in range(2):
                        nc.tensor.matmul(
                            out=po,
                            lhsT=attnT[jt][:, 128 * st:128 * (st + 1)],
                            rhs=woTb[:, jt, 512 * ec:512 * (ec + 1)],
                            start=(jt == 0), stop=(jt == 1))
                    if act_unload:
                        # Act is idle at the rep tail; unloading there frees
                        # the aux slots sooner for the next rep's projections
                        nc.scalar.activation(out=yst[:, 512 * ec:512 * (ec + 1)],
                                             in_=po, func=F.Copy)
                    else:
                        nc.vector.tensor_copy(out=yst[:, 512 * ec:512 * (ec + 1)],
                                              in_=po)
                # sync queue: the next rep's xt prefetch is issued at pair-0
                # mi 14, ahead of these in program order, so y can't delay it
                nc.sync.dma_start(out=y_out[128 * st:128 * (st + 1), :], in_=yst)

            # ---- attention pair (heads 2*jt, 2*jt+1) ----
            def attention_pair(jt, fill):
                pb = {0: 0, 1: 64}
                kTh, qTh = kTb[jt], qTb[jt]
                es = {}        # mip -> [128, 2(hpar), 2(sub), W_pair] fp16 tile
                ovs_hist = {}  # jg -> {hpar: psum tile}
                bcs = {}       # jg -> bc tile

                def pv_group(jg):
                    ovs = {}
                    for hpar in range(2):
                        h = 2 * jt + hpar
                        ov = ov_pool.tile([DK + 1, 512], FP32, tag=f"ov{hpar}",
                                          name=f"ov{jt}_{hpar}_{jg}")
                        ovs[hpar] = ov
                        for mi in range(0, 4 * jg + 4):
                            mip, sub = mi // 2, mi % 2
                            c0 = max(512 * jg, 128 * mi)   # abs col start
                            nc.tensor.matmul(
                                out=ov[:, c0 - 512 * jg:512],
                                lhsT=v_sb[:, mip, sub, h, 0:DK + 1],
                                rhs=es[mip][:, hpar, sub, c0 - 256 * mip:512 * (jg + 1) - 256 * mip],
                                start=(mi == 0), stop=(mi == 4 * jg + 3))
                    ovs_hist[jg] = ovs
                    bc = bc_pool.tile([128, 512], FP16, tag="bc", name=f"bc{jt}_{jg}")
                    bcs[jg] = bc
                    for hpar in range(2):
                        den = den_pool.tile([1, 512], FP16, tag="den",
                                            name=f"den{jt}_{hpar}_{jg}")
                        dend = dend_pool.tile([1, 512], FP16, tag="dend",
                                              name=f"dend{jt}_{hpar}_{jg}")
                        with nc.allow_low_precision(reason="1/den fits fp16"):
                            nc.vector.reciprocal(out=den, in_=ovs[hpar][DK:DK + 1, :])
                        # broadcast across partitions by bouncing through DRAM
                        # (a DRAM source may replicate; SBUF sources can't).
                        # scalar queue keeps it clear of xtb/swp/y on sync.
                        nc.scalar.dma_start(out=dend, in_=den)
                        nc.scalar.dma_start(
                            out=bc[pb[hpar]:pb[hpar] + DK, :],
                            in_=dend.to_broadcast((DK, 512)))

                def norm_group(jg):
                    # fused PSUM unload + normalize: attnT = ov * (1/den)
                    cs = slice(512 * jg, 512 * (jg + 1))
                    for hpar in range(2):
                        with nc.allow_low_precision(reason="attn weights fp16"):
                            nc.vector.tensor_mul(
                                attnT[jt][pb[hpar]:pb[hpar] + DK, cs],
                                ovs_hist[jg][hpar][0:DK, :],
                                bcs[jg][pb[hpar]:pb[hpar] + DK, :])

                for mi in range(NT):
                    mip, sub = mi // 2, mi % 2
                    W = S - 128 * mi
                    WP = S - 256 * mip
                    if sub == 0:
                        es[mip] = es_pool.tile(
                            [128, 2, 2, WP], FP16, tag=f"es{mip}",
                            name=f"es{jt}_{mip}")
                    for cb in range(0, W, 512):
                        cw = min(512, W - cb)
                        sp = sp_pool.tile([128, 2, 512], FP32, tag="sp",
                                          name=f"sp{jt}_{mi}_{cb}")
                        n0 = 128 * mi + cb
                        # the two heads' QK matmuls sit on disjoint PE row
                        # groups (0-63 / 64-127) and PSUM banks -> concurrent
                        for hpar in range(2):
                            nc.tensor.matmul(
                                out=sp[:, hpar, 0:cw],
                                lhsT=kTh[pb[hpar]:pb[hpar] + DK, 128 * mi:128 * (mi + 1)],
                                rhs=qTh[pb[hpar]:pb[hpar] + DK, n0:n0 + cw],
                                start=True, stop=True)
                        # one activation covers both heads' scores
                        with nc.allow_low_precision(reason="softmax weights fp16"):
                            nc.scalar.activation(
                                out=es[mip][:, :, sub, 128 * sub + cb:128 * sub + cb + cw],
                                in_=sp[:, :, 0:cw],
                                func=F.Exp, scale=SCALE, bias=expbias)
                    # causal mask on the diagonal 128 cols
                    for hpar in range(2):
                        dg = slice(128 * sub, 128 * sub + 128)
                        nc.gpsimd.affine_select(
                            out=es[mip][:, hpar, sub, dg],
                            in_=es[mip][:, hpar, sub, dg],
                            pattern=[[1, 128]], compare_op=A.is_ge, fill=0.0,
                            base=0, channel_multiplier=-1)
                    # staggered: pv at 4jg+4, normalize at 4jg+6
                    if mi in (4, 8, 12):
                        pv_group(mi // 4 - 1)
                    elif mi in (6, 10, 14):
                        norm_group((mi - 6) // 4)
                    for thunk in fill.get(mi, ()):
                        thunk()
                # tail: pv(3) -> recips/broadcast; fill['tail'] runs on PE
                # during that latency; then norm(3) -> fill['post']
                pv_group(3)
                for thunk in fill.get("tail", ()):
                    thunk()
                norm_group(3)
                for thunk in fill.get("post", ()):
                    thunk()

            # ---------------- rep body ----------------
            if _rep == 0:
                # later reps get k0/q0 prebuilt during the previous pair 1
                xtb_dma()
                for sc in range(NC):
                    proj_chunk("k", 0, sc)
                for sc in range(NC):
                    proj_chunk("q", 0, sc)
                proj_rope("k", 0)
                proj_rope("q", 0)

            fill0 = {
                0: [lambda: project_v(0), lambda: project_v(1)],
                1: [lambda: project_v(2), lambda: project_v(3)],
                2: [lambda: proj_chunk("k", 1, 0), lambda: proj_chunk("k", 1, 1)],
                3: [lambda: proj_chunk("k", 1, 2), lambda: proj_chunk("k", 1, 3)],
                4: [lambda: proj_rope("k", 1), lambda: project_v(4)],
                5: [lambda: project_v(5), lambda: proj_chunk("q", 1, 0)],
                6: [lambda: proj_chunk("q", 1, 1), lambda: proj_chunk("q", 1, 2)],
                7: [lambda: proj_chunk("q", 1, 3), lambda: project_v(6),
                    lambda: project_v(7)],
                8: [lambda: proj_rope("q", 1), lambda: project_v(8)],
                9: [lambda: project_v(9), lambda: project_v(10)],
                10: [lambda: project_v(11)],
                11: [lambda: project_v(12), lambda: project_v(13)],
                12: [lambda: project_v(14)],
                13: [lambda: project_v(15)],
                14: [xtb_dma],
            }
            attention_pair(0, fill0)

            fill1 = {
                0: [lambda: proj_chunk("k", 0, 0), lambda: proj_chunk("k", 0, 1)],
                1: [lambda: proj_chunk("k", 0, 2), lambda: proj_chunk("k", 0, 3)],
                2: [lambda: proj_rope("k", 0), lambda: proj_chunk("q", 0, 0)],
                3: [lambda: proj_chunk("q", 0, 1), lambda: proj_chunk("q", 0, 2)],
                4: [lambda: proj_chunk("q", 0, 3)],
                5: [lambda: proj_rope("q", 0)],
                7: [lambda: out_proj_st(0)],
                8: [lambda: out_proj_st(1)],
                9: [lambda: out_proj_st(2), lambda: out_proj_st(3)],
                11: [lambda: out_proj_st(4), lambda: out_proj_st(5)],
                12: [lambda: out_proj_st(6)],
                13: [lambda: out_proj_st(7)],
                15: [lambda: out_proj_st(8), lambda: out_proj_st(9)],
                "tail": [lambda: out_proj_st(10, True), lambda: out_proj_st(11, True)],
                "post": [lambda: out_proj_st(12, True), lambda: out_proj_st(13, True),
                         lambda: out_proj_st(14, True), lambda: out_proj_st(15, True)],
            }
            attention_pair(1, fill1)

        for p in (aux_pool, ov_pool, sp_pool, ysb, bc_pool, dend_pool,
                  den_pool, es_pool, ropep, persist):
            p.release()

    nc.compile()
    return nc


_cache = {}

def _get_program(S):
    if S not in _cache:
        _cache[S] = build_mha(S)
    return _cache[S]


def make_in_maps(x, token_positions, wq, wk, wv, wo):
    S = x.shape[1]
    f16 = np.float16
    invfreq = ROPE_THETA ** (-np.arange(0, DK, 2, dtype=np.float64) / DK)  # [32]
    # perm: within each 64-wide head block, evens first then odds
    blockperm = np.concatenate([np.arange(0, DK, 2), np.arange(1, DK, 2)])
    jperm = np.concatenate([64 * hh + blockperm for hh in range(HL)])

    pos = np.asarray(token_positions, dtype=np.float64)  # [B, S]
    tables = []
    for b in range(B):
        ang = pos[b][None, :] * invfreq[:, None]          # [32, S]
        cos = np.cos(ang)
        sin = np.sin(ang)
        cos64 = np.concatenate([cos, cos], axis=0).astype(f16)       # [64, S]
        sinalt = np.concatenate([-sin, sin], axis=0).astype(f16)     # [64, S]
        tables.append((np.ascontiguousarray(cos64), np.ascontiguousarray(sinalt)))

    in_maps = []
    for c in range(NCORES):
        b, g = c // GROUPS, c % GROUPS
        js = slice(JL * g, JL * (g + 1))
        cos64, sinalt = tables[b]
        in_maps.append({
            "xt": np.ascontiguousarray(x[b].T).astype(f16),
            "wqt": np.ascontiguousarray(wq[js, :][jperm, :].T).astype(f16),
            "wkt": np.ascontiguousarray(wk[js, :][jperm, :].T).astype(f16),
            "wvt": np.ascontiguousarray(wv[js, :].T).astype(f16),
            "wot": np.ascontiguousarray(wo[:, js].T).astype(f16),
            "cos64": cos64,
            "sinalt64": sinalt,
        })
    return in_maps


def kernel(x, token_positions, wq, wk, wv, wo):
    x = np.asarray(x, dtype=np.float32)
    token_positions = np.asarray(token_positions)
    wq = np.asarray(wq, dtype=np.float32)
    wk = np.asarray(wk, dtype=np.float32)
    wv = np.asarray(wv, dtype=np.float32)
    wo = np.asarray(wo, dtype=np.float32)
    S = x.shape[1]

    nc = _get_program(S)
    in_maps = make_in_maps(x, token_positions, wq, wk, wv, wo)
    res = run_bass_kernel_spmd(nc, in_maps, core_ids=list(range(NCORES)))
    out = np.zeros((B, S, D), dtype=np.float32)
    for c in range(NCORES):
        out[c // GROUPS] += res.results[c]["y"].astype(np.float32)
    return out
